# revision 14
# baseline (speedup 1.0000x reference)
"""Trainium2 Bass kernel for nn_Memory (GRU-style scan over 16384 rows, d=512).

Strategy: the recurrence m_t = (1-z_t) m_{t-1} + z_t h_t is *linear in m given
the gates*, and the gates depend on m_{t-1} through two 512x512 matvecs. We
solve each 2048-step block by fixed-point iteration: compute all gates from the
previous iterate's shifted states with large batched matmuls (full PE
utilization), then re-propagate the states exactly with the hardware linear
scan primitive (tensor_tensor_scan). ~16 passes converge to ~2e-3 max rel err
(fp16 matmul operands, fp32 accumulation and scan state).

Everything runs on core 0 in a single NEFF:
  phase 1: transpose x via PE, compute AZ^T/AH^T = (x@Wz)^T,(x@Wh)^T in fp16,
           stage to a DRAM scratch in [feat%128, chunk, t] layout.
  phase 2: per 2048-block: 17 gate/scan passes (pass 0 skips the U matmuls
           since the initial state guess is 0; last pass scans in fp32),
           then PE-transpose the block's states back to row layout -> ys.
"""

import sys

sys.path.insert(0, "/opt/trn_rl_repo")

import numpy as np

import concourse.bass as bass
import concourse.mybir as mybir
import concourse.tile as tile
from concourse.bass_utils import run_bass_kernel_spmd

T = 16384
D = 512  # in/out features
DO = 2 * D  # packed gate outputs (z | h)
B = 2048  # fixed-point block length
NBLK = T // B
NPASS = 17  # gate/scan passes per block (pass 0: no U-matmul; last: fp32 scan)
KCH = D // 128  # 4 contraction chunks
JCH = DO // 128  # 8 output chunks (0..3 -> z, 4..7 -> h)
NSUB = B // 512  # 512-column matmul subtiles per block

FP32 = mybir.dt.float32
FP16 = mybir.dt.float16
AF = mybir.ActivationFunctionType
ALU = mybir.AluOpType


def _apply_tile_drain_patch():
    """This container's walrus rejects >1 sync-wait on the TileContext exit
    Drain (setupSyncWait/CTRL_NO_STRUCT). Split the accumulated end-of-kernel
    waits into one Drain per semaphore."""
    import bass_rust

    def _drain_and_barrier(self, tick_clock, wait_clock):
        drain_inst = self.nc.sync.drain()
        wait_clock.add_sem_waits(
            drain_inst.ins, tile.ScopedClock({None: tick_clock.global_clock})
        )
        si = drain_inst.ins.sync_info
        if si is not None and len(si.on_wait) > 1:
            waits = list(si.on_wait)
            si.on_wait = waits[:1]
            for w in waits[1:]:
                d2 = self.nc.sync.drain()
                s2 = d2.ins.sync_info
                if s2 is None:
                    d2.ins.sync_info = bass_rust.SyncInfo(on_wait=[w], on_update=[])
                else:
                    s2.on_wait = [w]
        self.nc.all_engine_barrier()
        assert self.sems is not None
        popped = self.nc._tile_sem_poison_stack.pop()
        assert popped is self._sem_poison
        self.nc.clear_and_free_semaphores(list(self.sems.allocated().values()))
        self.nc.all_engine_barrier()

    tile.TileContext._drain_and_barrier = _drain_and_barrier


def _split_multi_waits(nc):
    """This walrus build encodes at most ONE sync-wait per hardware
    instruction. Hoist extra waits onto same-engine NoOps placed immediately
    before the owning instruction (engines execute block order, so the waits
    still all complete before it runs)."""
    import bass_rust

    nid = 0
    for f in nc.m.functions:
        for b in f.blocks:
            out = []
            changed = False
            for ins in b.instructions:
                si = ins.sync_info
                if si is not None and len(si.on_wait) > 1:
                    waits = list(si.on_wait)
                    for w in waits[:-1]:
                        nop = mybir.InstNoOp(name=f"I-waitsplit-{nid}", ins=[], outs=[])
                        nid += 1
                        nop.engine = ins.engine
                        nop.sync_info = bass_rust.SyncInfo(on_wait=[w], on_update=[])
                        out.append(nop)
                    si.on_wait = waits[-1:]
                    changed = True
                out.append(ins)
            if changed:
                b.instructions = out


def build_kernel():
    _apply_tile_drain_patch()
    nc = bass.Bass("TRN2")

    x = nc.dram_tensor("x", [T, D], FP32, kind="ExternalInput")
    wp = nc.dram_tensor("wp", [D, DO], FP16, kind="ExternalInput")  # [Wz|Wh]
    up = nc.dram_tensor("up", [D, DO], FP16, kind="ExternalInput")  # [Uz|Uh]
    i16 = nc.dram_tensor("i16", [128, 128], FP16, kind="ExternalInput")
    i32 = nc.dram_tensor("i32", [128, 128], FP32, kind="ExternalInput")
    bp = nc.dram_tensor("bp", [128, JCH], FP32, kind="ExternalInput")  # bias chunks
    ys = nc.dram_tensor("ys", [T, D], FP32, kind="ExternalOutput")

    with tile.TileContext(nc) as tc:
        consts = tc.alloc_tile_pool(name="consts", bufs=1)
        usb = consts.tile([128, KCH, DO], FP16, tag="usb")
        wsb = consts.tile([128, KCH, DO], FP16, tag="wsb")
        id16 = consts.tile([128, 128], FP16, tag="id16")
        id32 = consts.tile([128, 128], FP32, tag="id32")
        bsb = consts.tile([128, JCH], FP32, tag="bsb")
        nc.sync.dma_start(usb[:], up.rearrange("(k p) m -> p k m", p=128))
        nc.sync.dma_start(wsb[:], wp.rearrange("(k p) m -> p k m", p=128))
        nc.sync.dma_start(id16[:], i16[:])
        nc.sync.dma_start(id32[:], i32[:])
        nc.sync.dma_start(bsb[:], bp[:])

        dram = tc.alloc_tile_pool(name="dram", bufs=1, space="DRAM")
        # AZ^T/AH^T staged as [feat%128, out-chunk j, t]; j<4: z, j>=4: h
        azt = dram.tile([128, JCH, T], FP16, tag="azt")

        # ---------------- phase 1: x^T and AZ/AH ----------------
        with (
            tc.tile_pool(name="p1", bufs=3) as p1,
            tc.tile_pool(name="p1ps", bufs=4, space="PSUM") as p1ps,
            tc.tile_pool(name="p1az", bufs=2, space="PSUM") as p1az,
        ):
            for tb in range(T // 512):
                xT = p1.tile([128, KCH, 512], FP16, tag="xT")
                for s in range(4):
                    xt = p1.tile([128, D], FP32, tag="xt")
                    t0 = tb * 512 + s * 128
                    nc.sync.dma_start(xt[:], x[t0 : t0 + 128, :])
                    for k in range(KCH):
                        pst = p1ps.tile([128, 128], FP32, tag="pst")
                        nc.tensor.transpose(
                            pst[:], xt[:, k * 128 : (k + 1) * 128], id32[:]
                        )
                        nc.vector.tensor_copy(
                            xT[:, k, s * 128 : (s + 1) * 128], pst[:]
                        )
                az16 = p1.tile([128, JCH, 512], FP16, tag="az16")
                for j in range(JCH):
                    psa = p1az.tile([128, 512], FP32, tag="psa")
                    for k in range(KCH):
                        nc.tensor.matmul(
                            psa[:],
                            wsb[:, k, j * 128 : (j + 1) * 128],
                            xT[:, k, :],
                            start=(k == 0),
                            stop=(k == KCH - 1),
                        )
                    # az16 = psum + bias_chunk (per-partition), cast fp16
                    nc.scalar.activation(
                        az16[:, j, :], psa[:], AF.Identity, bias=bsb[:, j : j + 1]
                    )
                nc.sync.dma_start(azt[:, :, tb * 512 : (tb + 1) * 512], az16[:])

        # ---------------- phase 2: blockwise fixed point ----------------
        with (
            tc.tile_pool(name="st", bufs=1) as st,
            tc.tile_pool(name="gates", bufs=1) as gates,
            tc.tile_pool(name="az2", bufs=1) as az2,
            tc.tile_pool(name="carry", bufs=2) as carryp,
            tc.tile_pool(name="outs", bufs=4) as outs,
            tc.tile_pool(name="ps2", bufs=3, space="PSUM") as ps2,
            tc.tile_pool(name="pst2", bufs=2, space="PSUM") as pst2,
        ):
            carry = carryp.tile([128, KCH], FP32, tag="carry")
            nc.vector.memset(carry[:], 0.0)

            for b in range(NBLK):
                azb = az2.tile([128, JCH, B], FP16, tag="azb")
                nc.sync.dma_start(azb[:], azt[:, :, b * B : (b + 1) * B])

                # states, shifted by one: col 0 = carry, cols 1.. = m_t
                mx = st.tile([128, KCH, B + 1], FP16, tag="mx")
                m32 = st.tile([128, KCH, B], FP32, tag="m32")
                for c in range(KCH):
                    nc.vector.tensor_copy(mx[:, c, 0:1], carry[:, c : c + 1])

                zt = gates.tile([128, KCH, B], FP16, tag="zt")
                ht = gates.tile([128, KCH, B], FP16, tag="ht")
                d0 = gates.tile([128, KCH, B], FP16, tag="d0")
                d1 = gates.tile([128, KCH, B], FP16, tag="d1")

                for p in range(NPASS):
                    first = p == 0
                    last = p == NPASS - 1
                    for c in range(KCH):
                        for j in (c, c + KCH):  # z-chunk then h-chunk
                            for s in range(NSUB):
                                psg = ps2.tile([128, 512], FP32, tag="psg")
                                nc.tensor.matmul(
                                    psg[:],
                                    id16[:],
                                    azb[:, j, s * 512 : (s + 1) * 512],
                                    start=True,
                                    stop=first,
                                )
                                if not first:
                                    for k in range(KCH):
                                        nc.tensor.matmul(
                                            psg[:],
                                            usb[:, k, j * 128 : (j + 1) * 128],
                                            mx[:, k, s * 512 : s * 512 + 512],
                                            start=False,
                                            stop=(k == KCH - 1),
                                        )
                                dst = zt if j < KCH else ht
                                fn = AF.Sigmoid if j < KCH else AF.Tanh
                                nc.scalar.activation(
                                    dst[:, c, s * 512 : (s + 1) * 512], psg[:], fn
                                )
                        # d0 = 1 - z ; d1 = z * h
                        nc.vector.tensor_scalar(
                            d0[:, c, :], zt[:, c, :], -1.0, 1.0, ALU.mult, ALU.add
                        )
                        nc.vector.tensor_mul(d1[:, c, :], zt[:, c, :], ht[:, c, :])
                        # m_t = d0_t * m_{t-1} + d1_t  (exact sequential scan)
                        out_ap = m32[:, c, :] if last else mx[:, c, 1 : B + 1]
                        nc.vector.tensor_tensor_scan(
                            out_ap,
                            d0[:, c, :],
                            d1[:, c, :],
                            carry[:, c : c + 1],
                            ALU.mult,
                            ALU.add,
                        )

                ncarry = carryp.tile([128, KCH], FP32, tag="carry")
                for c in range(KCH):
                    nc.vector.tensor_copy(ncarry[:, c : c + 1], m32[:, c, B - 1 : B])
                carry = ncarry

                # transpose states back to [t, feat] rows and store
                for tt in range(B // 128):
                    yst = outs.tile([128, D], FP32, tag="yst")
                    for c in range(KCH):
                        psy = pst2.tile([128, 128], FP32, tag="psy")
                        nc.tensor.transpose(
                            psy[:], m32[:, c, tt * 128 : (tt + 1) * 128], id32[:]
                        )
                        nc.vector.tensor_copy(
                            yst[:, c * 128 : (c + 1) * 128], psy[:]
                        )
                    t0 = b * B + tt * 128
                    nc.sync.dma_start(ys[t0 : t0 + 128, :], yst[:])

        consts.release()
        dram.release()

    _split_multi_waits(nc)
    return nc


_CACHE = {}


def _make_runner(nc):
    """Single-core PJRT runner with a persistent jit cache (run_bass_via_pjrt
    builds a fresh closure per call, forcing a full recompile; this keeps the
    jitted body alive so repeat calls only pay transfer + execute)."""
    import jax
    from concourse import bass2jax

    bass2jax.install_neuronx_cc_hook()
    part_name = nc.partition_id_tensor.name if nc.partition_id_tensor else None
    in_names, out_names, out_avals = [], [], []
    for alloc in nc.m.functions[0].allocations:
        if not isinstance(alloc, mybir.MemoryLocationSet):
            continue
        name = alloc.memorylocations[0].name
        if alloc.kind == "ExternalInput":
            if name != part_name:
                in_names.append(name)
        elif alloc.kind == "ExternalOutput":
            out_names.append(name)
            out_avals.append(
                jax.core.ShapedArray(
                    tuple(alloc.tensor_shape), mybir.dt.np(alloc.dtype)
                )
            )
    n_params = len(in_names)
    all_names = in_names + out_names
    if part_name is not None:
        all_names = all_names + [part_name]
    all_names = tuple(all_names)
    donate = tuple(range(n_params, n_params + len(out_names)))

    def _body(*args):
        operands = list(args)
        if part_name is not None:
            operands.append(bass2jax.partition_id_tensor())
        outs = bass2jax._bass_exec_p.bind(
            *operands,
            out_avals=tuple(out_avals),
            in_names=all_names,
            out_names=tuple(out_names),
            lowering_input_output_aliases=(),
            sim_require_finite=True,
            sim_require_nnan=True,
            nc=nc,
        )
        return tuple(outs)

    jitted = jax.jit(_body, donate_argnums=donate, keep_unused=True)

    def run(in_map):
        args = [np.asarray(in_map[n]) for n in in_names[:n_params]]
        args += [np.zeros(a.shape, a.dtype) for a in out_avals]
        outs = jax.block_until_ready(jitted(*args))
        return {name: outs[i] for i, name in enumerate(out_names)}

    return run


def kernel(**inputs: np.ndarray) -> np.ndarray:
    run_kwargs = inputs.pop("_run_kwargs", {}) if "_run_kwargs" in inputs else {}
    x = np.ascontiguousarray(inputs["x"], dtype=np.float32)
    Wz = np.asarray(inputs["Wz"], dtype=np.float32)
    Uz = np.asarray(inputs["Uz"], dtype=np.float32)
    Wh = np.asarray(inputs["Wh"], dtype=np.float32)
    Uh = np.asarray(inputs["Uh"], dtype=np.float32)
    bz = np.asarray(inputs["bz"], dtype=np.float32)
    bh = np.asarray(inputs["bh"], dtype=np.float32)

    wp = np.concatenate([Wz, Wh], axis=1).astype(np.float16)
    up = np.concatenate([Uz, Uh], axis=1).astype(np.float16)
    bpack = np.concatenate([bz, bh]).reshape(JCH, 128).T.copy().astype(np.float32)
    eye16 = np.eye(128, dtype=np.float16)
    eye32 = np.eye(128, dtype=np.float32)

    if "nc" not in _CACHE:
        _CACHE["nc"] = build_kernel()
    nc = _CACHE["nc"]

    in_map = {
        "x": x,
        "wp": wp,
        "up": up,
        "i16": eye16,
        "i32": eye32,
        "bp": bpack,
    }
    import jax

    # Pin a real neuron device: with a CPU default device the bass_exec
    # primitive lowers to the MultiCoreSim fallback instead of hardware.
    dev = [d for d in jax.devices() if d.platform != "cpu"][0]
    if "runner" not in _CACHE:
        _CACHE["runner"] = _make_runner(nc)
    run = _CACHE["runner"]
    last_exc = None
    for attempt in range(3):
        try:
            with jax.default_device(dev):
                out = run(in_map)
            return np.ascontiguousarray(out["ys"])
        except Exception as e:  # transient NRT device errors on first exec
            last_exc = e
            if "UNRECOVERABLE" not in str(e) and "NRT" not in str(e):
                raise
    raise last_exc


if __name__ == "__main__":
    rng = np.random.RandomState(0)
    ins = {
        "x": rng.randn(T, D).astype(np.float32),
        "Wz": (rng.randn(D, D) / np.sqrt(D)).astype(np.float32),
        "Uz": (rng.randn(D, D) / np.sqrt(D)).astype(np.float32),
        "bz": np.zeros(D, np.float32),
        "Wh": (rng.randn(D, D) / np.sqrt(D)).astype(np.float32),
        "Uh": (rng.randn(D, D) / np.sqrt(D)).astype(np.float32),
        "bh": np.zeros(D, np.float32),
    }
    out = kernel(**ins)
    print("out", out.shape, out.dtype, np.abs(out).max())
